# revision 16
# baseline (speedup 1.0000x reference)
"""Trainium2 Bass kernel for nn_MinimalAttnHead.

Computes, per batch b:
    EW      = E @ W.T                       # [S, D]
    scores  = (E @ EW.T) / sqrt(D)          # [S, S]
    attn    = softmax(causal_mask(scores))  # causal: key k > query q masked
    out     = attn @ E                      # [S, D]

with B=4, S=4096, D=256, fp32 in/out.

Sharding: 8 cores = (batch b in 0..3) x (half h in 0..1). Queries of each
batch are split into 16 strips of 256; core half h=0 takes strips
{0,3,4,7,8,11,12,15}, h=1 the rest — causal work is exactly balanced.
Every core runs the SAME program; per-core data (query/key slices,
per-tile additive bias) encodes which strips it owns.

v2: the "past" key-tiles (the bulk of the work) run in fp8-e4m3 with
MatmulPerfMode.DoubleRow (2 contraction subtiles per instruction at 2x
rate: a [128,256]-score tile costs 113ns vs 257ns fp16, measured), and
the exp is batched: 4 key-tiles of scores land in one [128,1024] PSUM
tile (2 banks) -> ONE ACT exp per quad, amortizing the ~260ns ACT
instruction overhead.  The exp writes P directly as fp8, so the
P @ [V | ones] output matmuls are ALSO fp8 DoubleRow (2 key-tiles per
instruction).  The diagonal tiles stay fp16 end-to-end: the early rows
of each strip are dominated by diag keys and fp8 there breaks the 2e-2
tolerance (measured 2.2e-2 all-fp8 vs 9e-3 past-only-fp8 in numpy).

Per slot (s = core's strips sorted ascending) the program runs 1 diag
unit (2 key-tiles, one 384-col exp), PAIRS[s]//2 quad units (4 key-
tiles each), and 1 leftover pair unit (2 key-tiles).  Dead work (tiles
a strip doesn't need, kept for SPMD uniformity) is always exactly the
leftover pair, so quads share a constant exp bias and only the leftover
takes a per-core bias column (0 or -1e6, killing dead tiles inside the
ACT exp).  The PE stream is software-pipelined one unit deep.

Softmax is max-free (scores/16 ~ N(0,1)): P = exp(S/16 - 4.85); the
diag causal mask is applied multiplicatively to P's two masked
128-blocks on the otherwise-idle GpSimd.  Numerator and denominator
accumulate together in PSUM via a ones-column in V; the raw [num | den]
accumulators are copied out f16 and normalized HOST-side.

Engines: PE matmuls; ACT does ONLY exp (its table is preloaded by a
dummy exp at t=0); DVE does the XQ PSUM->SBUF casts (f16 + fp8), the
accumulator copies, and a few DMA triggers; Sync triggers the main DMA
stream; GpSimd does memsets + tri masks.
"""

import contextlib
import ctypes
import sys
import types

import numpy as np
import ml_dtypes

for _p in ("/opt/trn_rl_repo",):
    if _p not in sys.path:
        sys.path.insert(0, _p)

import concourse.bacc as bacc
import concourse.bass as bass
import concourse.mybir as mybir
import concourse.tile as tile
from concourse import bass_utils

# ---------------------------------------------------------------- constants
B, S, D = 4, 4096, 256
QSTRIP = 256                    # queries per strip
NSLOT = 8                       # strips per core
PAST = [2, 6, 10, 14, 18, 22, 26, 30]   # past k-tiles (of 128 keys) per slot
PAIRS = [p // 2 for p in PAST]          # past k-tile PAIRS per slot
PAST_KEYS = 128 * PAST[-1]      # 3840: keys ever read as "past"
STRIPS = {0: [0, 3, 4, 7, 8, 11, 12, 15], 1: [1, 2, 5, 6, 9, 10, 13, 14]}
NEG = -1.0e6                    # additive mask / bias value (exp -> 0)
EXP_BIAS = -4.85                # constant exp shift: num/den fit fp16
INV_SQRT_D = 1.0 / 16.0
NQ = NSLOT * QSTRIP             # 2048 queries per core
VW = D + 2                      # V row: D cols + ones + zero pad
NPT = PAST_KEYS // 128          # 30 past key tiles
NDG = QSTRIP // 128             # 2 diag key tiles per slot
NDT = NQ // 128                 # 16 diag key tiles total
NU = QSTRIP // 128              # 2 output 128-query groups per slot
NWU = 12                        # PE warm-up matmuls
NBIAS = NSLOT + 1               # per-slot leftover-pair bias + shared live col
# ETP col ranges (per half) needed newly by slot s's past tiles
ETP_CH = [(0, 128 * PAST[0])] + [
    (128 * PAST[s - 1], 128 * PAST[s]) for s in range(1, NSLOT)
]
# VP tile ranges per slot
VP_CH = [(0, PAST[0])] + [(PAST[s - 1], PAST[s]) for s in range(1, NSLOT)]
# xq chunk c covers ETQ cols [512c, 512(c+1)) = slots 2c, 2c+1; chunk c+1 is
# emitted mid-way through slot 2c+1's unit loop
XQ_AT = {1: (1, 1), 3: (2, 2), 5: (3, 3)}

F32 = mybir.dt.float32
F16 = mybir.dt.float16
F8 = mybir.dt.float8e4
NP8 = ml_dtypes.float8_e4m3
DR = mybir.MatmulPerfMode.DoubleRow

_CACHE = {}


# ------------------------------------------------------- axon NTFF trace shim
def _install_ntff_hook():
    """Provide antenv.axon_hooks (absent in this container) so
    run_bass_kernel_spmd(trace=True) can profile via libaxon_pjrt.so."""
    if "antenv.axon_hooks" in sys.modules:
        return
    try:
        import antenv
    except ImportError:
        return
    mod = types.ModuleType("antenv.axon_hooks")
    mod._hook = None
    mod.set_axon_ntff_profile_hook = lambda h: setattr(mod, "_hook", h)
    mod.get_axon_ntff_profile_hook = lambda: mod._hook
    sys.modules["antenv.axon_hooks"] = mod
    antenv.axon_hooks = mod
    try:
        lib = ctypes.CDLL("/opt/axon/libaxon_pjrt.so")
        lib.axon_start_nrt_profile.argtypes = [
            ctypes.POINTER(ctypes.c_int64),
            ctypes.c_size_t,
        ]
        lib.axon_start_nrt_profile.restype = ctypes.c_int64
        lib.axon_stop_nrt_profile.argtypes = [ctypes.c_char_p]
        lib.axon_stop_nrt_profile.restype = ctypes.c_int64
    except OSError:
        return

    @contextlib.contextmanager
    def _hook(output_dir, device_ids):
        import jax

        jax.devices()
        if device_ids:
            ids = (ctypes.c_int64 * len(device_ids))(*device_ids)
            rc = lib.axon_start_nrt_profile(ids, len(device_ids))
        else:
            rc = lib.axon_start_nrt_profile(None, 0)
        if rc != 0:
            raise RuntimeError(f"axon_start_nrt_profile rc={rc}")
        try:
            yield
        finally:
            lib.axon_stop_nrt_profile(str(output_dir).encode())

    mod._hook = _hook
    # artifact upload needs monorepo fish paths; keep traces local
    bass_utils.upload_artifacts = lambda tmpdir: "local://" + tmpdir


# ------------------------------------------------------------- program build
def _build():
    nc = bacc.Bacc("TRN2", target_bir_lowering=False, debug=False)

    # all inputs already in SBUF-native [128, free] layout (host packs them)
    etq_d = nc.dram_tensor("ETQ", [128, 2 * NQ], F16, kind="ExternalInput")
    etp_d = nc.dram_tensor("ETP8", [128, 2 * PAST_KEYS], F8,
                           kind="ExternalInput")
    vp_d = nc.dram_tensor("VP8", [128, NPT * VW], F8, kind="ExternalInput")
    vd_d = nc.dram_tensor("VD", [128, NDT * VW], F16, kind="ExternalInput")
    w_d = nc.dram_tensor("W", [128, 2 * D], F16, kind="ExternalInput")
    tri_d = nc.dram_tensor("TRI", [128, 128], F16, kind="ExternalInput")
    bias_d = nc.dram_tensor("BIAS", [128, NBIAS], F32, kind="ExternalInput")
    # raw accumulators (num cols 0:256, den col 256); host divides.  The
    # exp carries a constant -4.85 bias so num/den fit fp16 (den<=63,
    # |num|<=20), halving the output stores; normalization cancels it.
    out_d = nc.dram_tensor("OUT", [128, NDT * VW], F16, kind="ExternalOutput")

    with tile.TileContext(nc) as tc:
        with (
            tc.tile_pool(name="persist", bufs=1) as pp,
            tc.tile_pool(name="psQ", bufs=2, space=bass.MemorySpace.PSUM) as psQ,
            tc.tile_pool(name="psP", bufs=2, space=bass.MemorySpace.PSUM) as psP,
            tc.tile_pool(name="psO", bufs=1, space=bass.MemorySpace.PSUM) as psO,
            tc.tile_pool(name="pwork", bufs=3) as wp,
            tc.tile_pool(name="swork", bufs=8) as sp,
        ):
            # ---------------- persistent SBUF ----------------
            w_sb = pp.tile([128, 2 * D], F16, tag="w", name="w")
            etq_sb = pp.tile([128, 2 * NQ], F16, tag="etq", name="etq")
            etp_sb = pp.tile([128, 2 * PAST_KEYS], F8, tag="etp", name="etp")
            xq_sb = pp.tile([128, 2 * NQ], F16, tag="xq", name="xq")
            xq8_sb = pp.tile([128, 2 * NQ], F8, tag="xq8", name="xq8")
            vp_sb = pp.tile([128, NPT * VW], F8, tag="vp", name="vp")
            vd_sb = pp.tile([128, NDT * VW], F16, tag="vd", name="vd")
            tri_sb = pp.tile([128, 128], F16, tag="tri", name="tri")
            bias_sb = pp.tile([128, NBIAS], F32, tag="bias", name="bias")

            def eth(t, h, c0, c1, n):
                return t[:, h * n + c0 : h * n + c1]

            def e3(t, n, c0, c1):
                """[128, 2, c1-c0] view of an h-major [128, 2n] tile."""
                return t[:].rearrange("p (h n) -> p h n", h=2)[:, :, c0:c1]

            def load_vcols(dst, src_t, c0, c1, engine):
                engine.dma_start(dst[:, c0:c1], src_t.ap()[:, c0:c1])

            def load_2h(dst, src_t, n, c0, c1, engine):
                engine.dma_start(
                    dst[:].rearrange("p (h n) -> p h n", h=2)[:, :, c0:c1],
                    src_t.ap().rearrange("p (h n) -> p h n", h=2)[:, :, c0:c1],
                )

            # PE warm-up: dense dummy matmuls on a memset tile, issued
            # before any load lands, so the HAM clock throttle ramps toward
            # full rate while the first DMAs stream in.  A dummy exp on the
            # same tile forces the ACT Exp table load off the critical path.
            wusrc = pp.tile([128, 512], F16, tag="wusrc", name="wusrc")
            zbias = pp.tile([128, 1], F32, tag="zbias", name="zbias")
            nc.vector.memset(wusrc[:], 1.0)
            nc.gpsimd.memset(zbias[:], 0.0)
            pre_t = wp.tile([128, 2], F16, tag="pre", name="pre")
            nc.scalar.activation(
                pre_t[:], wusrc[:, 0:2], mybir.ActivationFunctionType.Exp,
                bias=zbias[:, 0:1], scale=1.0,
            )
            for _ in range(NWU):
                wu = psP.tile([128, 512], F32, tag="st", name="wu")
                nc.tensor.matmul(
                    wu[:], wusrc[:, 0:128], wusrc[:],
                    start=True, stop=True,
                )

            # -------- loads, ordered by first-use; two trigger queues ------
            # Sync: chunk-0/1 ETQ + per-slot V/ETP.  Vector: W + mask/bias
            # + ETQ chunks 2-3 + slot-1 V (DVE's casts only start once
            # chunk 0 lands, so its early queue is free).  ACT: nothing.
            def load_1h(dst, src_t, n, h, c0, c1, engine):
                engine.dma_start(
                    dst[:, h * n + c0 : h * n + c1],
                    src_t.ap()[:, h * n + c0 : h * n + c1],
                )

            # The first compute (xq matmuls, then diag) is gated by W-h0 +
            # ETQ chunk-0-h0: split those loads into halves so each DMA is
            # small (lands sooner) and the halves run on parallel engines.
            nc.gpsimd.dma_start(w_sb[:, 0:D], w_d.ap()[:, 0:D])      # W h0
            load_1h(etq_sb, etq_d, NQ, 0, 0, 256, nc.sync)          # c0a h0
            nc.gpsimd.dma_start(w_sb[:, D : 2 * D],
                                w_d.ap()[:, D : 2 * D])              # W h1
            load_1h(etq_sb, etq_d, NQ, 1, 0, 256, nc.sync)          # c0a h1
            load_1h(etq_sb, etq_d, NQ, 0, 256, 512, nc.sync)        # c0b h0
            nc.gpsimd.dma_start(bias_sb[:], bias_d.ap())
            load_1h(etq_sb, etq_d, NQ, 1, 256, 512, nc.sync)        # c0b h1
            load_vcols(vd_sb, vd_d, 0, NDG * VW, nc.gpsimd)         # diag V s0
            nc.gpsimd.dma_start(tri_sb[:], tri_d.ap())
            load_vcols(vp_sb, vp_d, VP_CH[0][0] * VW, VP_CH[0][1] * VW,
                       nc.gpsimd)                                   # past V s0
            load_2h(etq_sb, etq_d, NQ, 512, 1024, nc.sync)          # chunk 1
            load_2h(etp_sb, etp_d, PAST_KEYS, *ETP_CH[0], nc.sync)  # past c0
            # slot-1 V + remaining ETQ chunks on GpSimd: Sync's early queue
            # is saturated with chunk-0/1 tensors
            load_vcols(vd_sb, vd_d, NDG * VW, 2 * NDG * VW, nc.gpsimd)
            load_vcols(vp_sb, vp_d, VP_CH[1][0] * VW, VP_CH[1][1] * VW,
                       nc.gpsimd)
            load_2h(etq_sb, etq_d, NQ, 1024, 1536, nc.gpsimd)       # chunk 2
            load_2h(etq_sb, etq_d, NQ, 1536, 2048, nc.gpsimd)       # chunk 3
            load_2h(etp_sb, etp_d, PAST_KEYS, *ETP_CH[1], nc.sync)
            for s in range(2, NSLOT):
                load_vcols(vd_sb, vd_d, s * NDG * VW, (s + 1) * NDG * VW,
                           nc.sync)
                load_2h(etp_sb, etp_d, PAST_KEYS, *ETP_CH[s], nc.sync)
                load_vcols(vp_sb, vp_d, VP_CH[s][0] * VW, VP_CH[s][1] * VW,
                           nc.sync)

            # XQ[d, q] = sum_e W[e, d] * ETQ[e, q] over query cols [qa, qb),
            # copied out both f16 (diag matmuls) and fp8 (past matmuls)
            def xq_range(qa, qb):
                # f16 casts first (they gate the next diag unit); the fp8
                # casts re-read the f16 SBUF copy so the PSUM tile frees
                # early and the diag path never queues behind them
                for dh in range(2):
                    ps = psP.tile([128, 512], F32, tag="st", name="st")
                    nc.tensor.matmul(
                        ps[:, 0 : qb - qa],
                        eth(w_sb, 0, dh * 128, (dh + 1) * 128, D),
                        eth(etq_sb, 0, qa, qb, NQ),
                        start=True,
                        stop=False,
                    )
                    nc.tensor.matmul(
                        ps[:, 0 : qb - qa],
                        eth(w_sb, 1, dh * 128, (dh + 1) * 128, D),
                        eth(etq_sb, 1, qa, qb, NQ),
                        start=False,
                        stop=True,
                    )
                    nc.vector.tensor_copy(
                        eth(xq_sb, dh, qa, qb, NQ), ps[:, 0 : qb - qa]
                    )
                for dh in range(2):
                    nc.vector.tensor_copy(
                        eth(xq8_sb, dh, qa, qb, NQ),
                        eth(xq_sb, dh, qa, qb, NQ),
                    )

            # ---------------- attention ----------------
            # Cross-slot streaming pipeline: `groups` carries
            # (matmul-args, callback) entries ACROSS slot boundaries; unit
            # j's out-matmuls are emitted right after unit j+1's score
            # matmuls, and each slot's accumulator-copy epilogue is emitted
            # as soon as its last out-matmul has actually been issued.
            groups = []

            def flush_groups(keep=0):
                while len(groups) > keep:
                    args_list, cb = groups.pop(0)
                    for args in args_list:
                        nc.tensor.matmul(*args[:3], start=args[3],
                                         stop=args[4], perf_mode=args[5])
                    if cb is not None:
                        cb()

            def mk_epilogue(s, outp):
                # copy the raw accumulators (num + den column) PSUM->SBUF,
                # store f16; host divides.
                def cb():
                    osb = sp.tile([128, NU * VW], F16, tag="osb", name="osb")
                    for u in range(NU):
                        nc.vector.tensor_copy(
                            osb[:, u * VW : (u + 1) * VW], outp[u][:]
                        )
                    nc.sync.dma_start(
                        out_d.ap()[:, s * NU * VW : (s + 1) * NU * VW],
                        osb[:],
                    )
                return cb

            # chunk 0 in two halves: slot 0 only needs XQ cols [0, 256), so
            # compute starts as soon as the first half-chunk of ETQ lands
            xq_range(0, 256)
            xq_range(256, 512)
            for s in range(NSLOT):
                q0 = s * QSTRIP
                outp = [
                    psO.tile([128, VW], F32, tag=f"outp{u}", name=f"outp{u}")
                    for u in range(NU)
                ]
                nquad = PAIRS[s] // 2
                # units: diag (2 k-tiles, fp16), quads (4 k-tiles, fp8),
                # leftover pair (2 k-tiles, fp8).  Last slot runs the diag
                # LAST so the copy/store epilogue overlaps its matmuls.
                # interleave: in quad-dense stretches ACT binds (quad exp
                # ~1.1us vs ~0.9us of PE work per quad), so the diag unit's
                # PE-heavy work goes AFTER the first quad to absorb a bubble
                if s == NSLOT - 1:
                    units = [("q", i) for i in range(nquad)] + [("p", 0),
                                                                ("d", 0)]
                elif nquad >= 1:
                    units = [("q", 0), ("d", 0)] + [
                        ("q", i) for i in range(1, nquad)] + [("p", 0)]
                else:
                    units = [("d", 0), ("p", 0)]
                pending = []
                started = [False, False]

                def out_mm(u, stat, mov, ustop):
                    pending.append((outp[u][:], stat, mov,
                                    not started[u], ustop, None))
                    started[u] = True

                def out_mm8(u, stat, mov, ustop):
                    pending.append((outp[u][:], stat, mov,
                                    not started[u], ustop, DR))
                    started[u] = True

                for j, (kind, idx) in enumerate(units):
                    last_unit = j == len(units) - 1
                    if kind == "d":
                        # ---- diag unit: 2 fp16 k-tiles, one 384-col exp
                        ps = psP.tile([128, 512], F32, tag="st", name="st")
                        for kt in range(NDG):
                            dt_i = s * NDG + kt
                            kcol = dt_i * 128
                            lo = kt * 128
                            c0 = kt * 256
                            for dh in range(2):
                                nc.tensor.matmul(
                                    ps[:, c0 : c0 + QSTRIP - lo],
                                    eth(etq_sb, dh, kcol, kcol + 128, NQ),
                                    eth(xq_sb, dh, q0 + lo, q0 + QSTRIP, NQ),
                                    start=dh == 0,
                                    stop=dh == 1,
                                )
                        flush_groups(keep=1)
                        p_t = wp.tile([128, 384], F16, tag="Pd", name="pd")
                        nc.scalar.activation(
                            p_t[:], ps[:, 0:384],
                            mybir.ActivationFunctionType.Exp,
                            bias=bias_sb[:, NSLOT : NSLOT + 1],
                            scale=INV_SQRT_D,
                        )
                        # zero the below-diagonal blocks multiplicatively on
                        # the otherwise-idle GpSimd; those out-matmuls go
                        # last so GpSimd has slack.
                        nc.gpsimd.tensor_mul(
                            p_t[:, 0:128], p_t[:, 0:128], tri_sb[:]
                        )
                        nc.gpsimd.tensor_mul(
                            p_t[:, 256:384], p_t[:, 256:384], tri_sb[:]
                        )
                        vt0 = vd_sb[:, (s * NDG) * VW : (s * NDG + 1) * VW]
                        vt1 = vd_sb[:, (s * NDG + 1) * VW : (s * NDG + 2) * VW]
                        last_s = s == NSLOT - 1
                        out_mm(1, p_t[:, 128:256], vt0, False)
                        out_mm(0, p_t[:, 0:128], vt0, last_s)
                        out_mm(1, p_t[:, 256:384], vt1, last_s)
                    elif kind == "q":
                        # ---- quad unit: pairs (2*idx, 2*idx+1) = 4 k-tiles
                        ps = psQ.tile([128, 1024], F32, tag="q", name="q")
                        for i in range(4):
                            kt = 4 * idx + i
                            nc.tensor.matmul(
                                ps[:, i * 256 : (i + 1) * 256],
                                e3(etp_sb, PAST_KEYS, kt * 128,
                                   (kt + 1) * 128),
                                e3(xq8_sb, NQ, q0, q0 + QSTRIP),
                                start=True, stop=True, perf_mode=DR,
                            )
                        flush_groups(keep=1)
                        p8 = wp.tile([128, 1024], F8, tag="Pq", name="pq")
                        nc.scalar.activation(
                            p8[:], ps[:],
                            mybir.ActivationFunctionType.Exp,
                            bias=bias_sb[:, NSLOT : NSLOT + 1],
                            scale=INV_SQRT_D,
                        )
                        for half in range(2):      # pair jp = 2*idx + half
                            kt0 = (2 * idx + half) * 2
                            mov = vp_sb[:, kt0 * VW : (kt0 + 2) * VW
                                        ].rearrange("p (h n) -> p h n", h=2)
                            pv = p8[:, half * 512 : (half + 1) * 512
                                    ].rearrange("p (h n) -> p h n", h=2)
                            for u in range(NU):
                                out_mm8(u, pv[:, :, u * 128 : (u + 1) * 128],
                                        mov, False)
                    else:
                        # ---- leftover pair: k-tiles (2jp, 2jp+1), per-core
                        # bias kills it on the half that doesn't need it
                        jp = PAIRS[s] - 1
                        ps = psP.tile([128, 512], F32, tag="st", name="st")
                        for i in range(2):
                            kt = 2 * jp + i
                            nc.tensor.matmul(
                                ps[:, i * 256 : (i + 1) * 256],
                                e3(etp_sb, PAST_KEYS, kt * 128,
                                   (kt + 1) * 128),
                                e3(xq8_sb, NQ, q0, q0 + QSTRIP),
                                start=True, stop=True, perf_mode=DR,
                            )
                        flush_groups(keep=1)
                        p8 = wp.tile([128, 512], F8, tag="Pp", name="pp8")
                        nc.scalar.activation(
                            p8[:], ps[:],
                            mybir.ActivationFunctionType.Exp,
                            bias=bias_sb[:, s : s + 1],
                            scale=INV_SQRT_D,
                        )
                        mov = vp_sb[:, 2 * jp * VW : (2 * jp + 2) * VW
                                    ].rearrange("p (h n) -> p h n", h=2)
                        pv = p8[:].rearrange("p (h n) -> p h n", h=2)
                        ustop = s != NSLOT - 1 and last_unit
                        for u in range(NU):
                            out_mm8(u, pv[:, :, u * 128 : (u + 1) * 128],
                                    mov, ustop)

                    groups.append((list(pending), None))
                    pending.clear()

                    # overlap a later slot-pair's XQ with this unit loop
                    if s in XQ_AT and j == XQ_AT[s][0]:
                        flush_groups(0)
                        c = XQ_AT[s][1]
                        xq_range(512 * c, 512 * (c + 1))

                # the slot's copy/store epilogue rides on its last unit
                groups[-1] = (groups[-1][0], mk_epilogue(s, outp))
            flush_groups(0)

    nc.compile()
    return nc


def _get_program():
    if "nc" not in _CACHE:
        _CACHE["nc"] = _build()
    return _CACHE["nc"]


# ------------------------------------------------------------- host-side data
def _hw2(x):
    """[256, N] -> SBUF-native [128, 2N] (halves side by side)."""
    return np.concatenate([x[0:128], x[128:256]], axis=1)


def _hwtiles(x):
    """[(n*128), v] -> SBUF-native [128, n*v]."""
    n = x.shape[0] // 128
    return np.ascontiguousarray(
        x.reshape(n, 128, x.shape[1]).transpose(1, 0, 2).reshape(128, -1)
    )


def _static_inputs():
    """TRI / BIAS-per-half: identical across calls."""
    if "static" in _CACHE:
        return _CACHE["static"]
    # multiplicative causal mask for the leading diag 128-block of P
    tri = np.where(
        np.arange(128)[:, None] <= np.arange(128)[None, :], 1.0, 0.0
    ).astype(np.float16)
    biases = {}
    for h in (0, 1):
        bias = np.full((128, NBIAS), EXP_BIAS, np.float32)
        for s, p in enumerate(sorted(STRIPS[h])):
            # leftover pair jp = PAIRS[s]-1 is live iff jp < p
            if not (PAIRS[s] - 1 < p):
                bias[:, s] = NEG
        biases[h] = bias
    _CACHE["static"] = (tri, biases)
    return _CACHE["static"]


def _core_inputs(encodings, W):
    tri, biases = _static_inputs()
    w = _hw2(np.asarray(W, np.float16))
    ones = np.ones((1,), NP8)
    pad = np.zeros((1,), NP8)
    in_maps = []
    for c in range(8):
        b, h = c // 2, c % 2
        e = np.asarray(encodings[b], np.float16)          # [S, D]
        et = np.ascontiguousarray(e.T)                    # [D, S]
        e8 = np.asarray(encodings[b], NP8)
        et8 = np.ascontiguousarray(e8.T)
        strips = sorted(STRIPS[h])
        etq = np.concatenate(
            [et[:, p * QSTRIP : (p + 1) * QSTRIP] for p in strips], axis=1
        )
        rows_d = np.concatenate(
            [e[p * QSTRIP : (p + 1) * QSTRIP] for p in strips], axis=0
        )
        ones16 = np.ones((1,), np.float16)
        pad16 = np.zeros((1,), np.float16)
        vd = np.concatenate(
            [rows_d, np.broadcast_to(ones16, (NQ, 1)),
             np.broadcast_to(pad16, (NQ, 1))], axis=1
        )
        vp8 = np.concatenate(
            [e8[:PAST_KEYS], np.broadcast_to(ones, (PAST_KEYS, 1)),
             np.broadcast_to(pad, (PAST_KEYS, 1))], axis=1
        )
        in_maps.append(
            {
                "ETQ": np.ascontiguousarray(_hw2(etq)),
                "ETP8": np.ascontiguousarray(_hw2(et8[:, :PAST_KEYS])),
                "VP8": _hwtiles(vp8),
                "VD": _hwtiles(vd),
                "W": np.ascontiguousarray(w),
                "TRI": tri,
                "BIAS": biases[h],
            }
        )
    return in_maps


def run_on_cores(encodings, W, trace=False, trace_cores=None):
    """Build+run; returns (output [B,S,D], BassKernelResults)."""
    _install_ntff_hook()
    nc = _get_program()
    in_maps = _core_inputs(encodings, W)
    res = bass_utils.run_bass_kernel_spmd(
        nc,
        in_maps,
        core_ids=list(range(8)),
        trace=trace,
        trace_cores=trace_cores,
    )
    out = np.empty((B, S, D), np.float32)
    for c in range(8):
        b, h = c // 2, c % 2
        o = res.results[c]["OUT"].astype(np.float32)      # [128, 16*258] f16
        o = o.reshape(128, NDT, VW).transpose(1, 0, 2).reshape(NQ, VW)
        o = o[:, 0:D] / o[:, D : D + 1]                   # host normalize
        for s, p in enumerate(sorted(STRIPS[h])):
            out[b, p * QSTRIP : (p + 1) * QSTRIP, :] = o[
                s * QSTRIP : (s + 1) * QSTRIP
            ]
    return out, res


def kernel(encodings, W):
    out, _ = run_on_cores(encodings, W, trace=False)
    return out


# revision 17
# speedup vs baseline: 1.0791x; 1.0791x over previous
"""Trainium2 Bass kernel for nn_MinimalAttnHead.

Computes, per batch b:
    EW      = E @ W.T                       # [S, D]
    scores  = (E @ EW.T) / sqrt(D)          # [S, S]
    attn    = softmax(causal_mask(scores))  # causal: key k > query q masked
    out     = attn @ E                      # [S, D]

with B=4, S=4096, D=256, fp32 in/out.

Sharding: 8 cores = (batch b in 0..3) x (half h in 0..1). Queries of each
batch are split into 16 strips of 256; core half h=0 takes strips
{0,3,4,7,8,11,12,15}, h=1 the rest — causal work is exactly balanced.
Every core runs the SAME program; per-core data (query/key slices,
per-tile additive bias) encodes which strips it owns.

v2: the "past" key-tiles (the bulk of the work) run in fp8-e4m3 with
MatmulPerfMode.DoubleRow (2 contraction subtiles per instruction at 2x
rate: a [128,256]-score tile costs 113ns vs 257ns fp16, measured), and
the exp is batched: 4 key-tiles of scores land in one [128,1024] PSUM
tile (2 banks) -> ONE ACT exp per quad, amortizing the ~260ns ACT
instruction overhead.  The exp writes P directly as fp8, so the
P @ [V | ones] output matmuls are ALSO fp8 DoubleRow (2 key-tiles per
instruction).  The diagonal tiles stay fp16 end-to-end: the early rows
of each strip are dominated by diag keys and fp8 there breaks the 2e-2
tolerance (measured 2.2e-2 all-fp8 vs 9e-3 past-only-fp8 in numpy).

Per slot (s = core's strips sorted ascending) the program runs 1 diag
unit (2 key-tiles, one 384-col exp), PAIRS[s]//2 quad units (4 key-
tiles each), and 1 leftover pair unit (2 key-tiles).  Dead work (tiles
a strip doesn't need, kept for SPMD uniformity) is always exactly the
leftover pair, so quads share a constant exp bias and only the leftover
takes a per-core bias column (0 or -1e6, killing dead tiles inside the
ACT exp).  The PE stream is software-pipelined one unit deep.

Softmax is max-free (scores/16 ~ N(0,1)): P = exp(S/16 - 4.85); the
diag causal mask is applied multiplicatively to P's two masked
128-blocks on the otherwise-idle GpSimd.  Numerator and denominator
accumulate together in PSUM via a ones-column in V; the raw [num | den]
accumulators are copied out f16 and normalized HOST-side.

Engines: PE matmuls; ACT does ONLY exp (its table is preloaded by a
dummy exp at t=0); DVE does the XQ PSUM->SBUF casts (f16 + fp8), the
accumulator copies, and a few DMA triggers; Sync triggers the main DMA
stream; GpSimd does memsets + tri masks.
"""

import contextlib
import ctypes
import sys
import types

import numpy as np
import ml_dtypes

for _p in ("/opt/trn_rl_repo",):
    if _p not in sys.path:
        sys.path.insert(0, _p)

import concourse.bacc as bacc
import concourse.bass as bass
import concourse.mybir as mybir
import concourse.tile as tile
from concourse import bass_utils

# ---------------------------------------------------------------- constants
B, S, D = 4, 4096, 256
QSTRIP = 256                    # queries per strip
NSLOT = 8                       # strips per core
PAST = [2, 6, 10, 14, 18, 22, 26, 30]   # past k-tiles (of 128 keys) per slot
PAIRS = [p // 2 for p in PAST]          # past k-tile PAIRS per slot
PAST_KEYS = 128 * PAST[-1]      # 3840: keys ever read as "past"
STRIPS = {0: [0, 3, 4, 7, 8, 11, 12, 15], 1: [1, 2, 5, 6, 9, 10, 13, 14]}
NEG = -1.0e6                    # additive mask / bias value (exp -> 0)
EXP_BIAS = -4.85                # constant exp shift: num/den fit fp16
INV_SQRT_D = 1.0 / 16.0
NQ = NSLOT * QSTRIP             # 2048 queries per core
VW = D + 2                      # V row: D cols + ones + zero pad
NPT = PAST_KEYS // 128          # 30 past key tiles
NDG = QSTRIP // 128             # 2 diag key tiles per slot
NDT = NQ // 128                 # 16 diag key tiles total
NU = QSTRIP // 128              # 2 output 128-query groups per slot
NWU = 12                        # PE warm-up matmuls
NBIAS = NSLOT + 1               # per-slot leftover-pair bias + shared live col
# ETP col ranges (per half) needed newly by slot s's past tiles
ETP_CH = [(0, 128 * PAST[0])] + [
    (128 * PAST[s - 1], 128 * PAST[s]) for s in range(1, NSLOT)
]
# VP tile ranges per slot
VP_CH = [(0, PAST[0])] + [(PAST[s - 1], PAST[s]) for s in range(1, NSLOT)]
# xq chunk c covers ETQ cols [512c, 512(c+1)) = slots 2c, 2c+1; chunk c+1 is
# emitted mid-way through slot 2c+1's unit loop
XQ_AT = {1: (1, 1), 3: (2, 2), 5: (3, 3)}

F32 = mybir.dt.float32
F16 = mybir.dt.float16
F8 = mybir.dt.float8e4
NP8 = ml_dtypes.float8_e4m3
DR = mybir.MatmulPerfMode.DoubleRow

_CACHE = {}


# ------------------------------------------------------- axon NTFF trace shim
def _install_ntff_hook():
    """Provide antenv.axon_hooks (absent in this container) so
    run_bass_kernel_spmd(trace=True) can profile via libaxon_pjrt.so."""
    if "antenv.axon_hooks" in sys.modules:
        return
    try:
        import antenv
    except ImportError:
        return
    mod = types.ModuleType("antenv.axon_hooks")
    mod._hook = None
    mod.set_axon_ntff_profile_hook = lambda h: setattr(mod, "_hook", h)
    mod.get_axon_ntff_profile_hook = lambda: mod._hook
    sys.modules["antenv.axon_hooks"] = mod
    antenv.axon_hooks = mod
    try:
        lib = ctypes.CDLL("/opt/axon/libaxon_pjrt.so")
        lib.axon_start_nrt_profile.argtypes = [
            ctypes.POINTER(ctypes.c_int64),
            ctypes.c_size_t,
        ]
        lib.axon_start_nrt_profile.restype = ctypes.c_int64
        lib.axon_stop_nrt_profile.argtypes = [ctypes.c_char_p]
        lib.axon_stop_nrt_profile.restype = ctypes.c_int64
    except OSError:
        return

    @contextlib.contextmanager
    def _hook(output_dir, device_ids):
        import jax

        jax.devices()
        if device_ids:
            ids = (ctypes.c_int64 * len(device_ids))(*device_ids)
            rc = lib.axon_start_nrt_profile(ids, len(device_ids))
        else:
            rc = lib.axon_start_nrt_profile(None, 0)
        if rc != 0:
            raise RuntimeError(f"axon_start_nrt_profile rc={rc}")
        try:
            yield
        finally:
            lib.axon_stop_nrt_profile(str(output_dir).encode())

    mod._hook = _hook
    # artifact upload needs monorepo fish paths; keep traces local
    bass_utils.upload_artifacts = lambda tmpdir: "local://" + tmpdir


# ------------------------------------------------------------- program build
def _build():
    nc = bacc.Bacc("TRN2", target_bir_lowering=False, debug=False)

    # all inputs already in SBUF-native [128, free] layout (host packs them)
    etq_d = nc.dram_tensor("ETQ", [128, 2 * NQ], F16, kind="ExternalInput")
    etp_d = nc.dram_tensor("ETP8", [128, 2 * PAST_KEYS], F8,
                           kind="ExternalInput")
    vp_d = nc.dram_tensor("VP8", [128, NPT * VW], F8, kind="ExternalInput")
    vd_d = nc.dram_tensor("VD", [128, NDT * VW], F16, kind="ExternalInput")
    w_d = nc.dram_tensor("W", [128, 2 * D], F16, kind="ExternalInput")
    tri_d = nc.dram_tensor("TRI", [128, 128], F16, kind="ExternalInput")
    bias_d = nc.dram_tensor("BIAS", [128, NBIAS], F32, kind="ExternalInput")
    # raw accumulators (num cols 0:256, den col 256); host divides.  The
    # exp carries a constant -4.85 bias so num/den fit fp16 (den<=63,
    # |num|<=20), halving the output stores; normalization cancels it.
    out_d = nc.dram_tensor("OUT", [128, NDT * VW], F16, kind="ExternalOutput")

    with tile.TileContext(nc) as tc:
        with (
            tc.tile_pool(name="persist", bufs=1) as pp,
            tc.tile_pool(name="psQ", bufs=2, space=bass.MemorySpace.PSUM) as psQ,
            tc.tile_pool(name="psP", bufs=2, space=bass.MemorySpace.PSUM) as psP,
            tc.tile_pool(name="psO", bufs=1, space=bass.MemorySpace.PSUM) as psO,
            tc.tile_pool(name="pwork", bufs=3) as wp,
            tc.tile_pool(name="swork", bufs=8) as sp,
        ):
            # ---------------- persistent SBUF ----------------
            w_sb = pp.tile([128, 2 * D], F16, tag="w", name="w")
            etq_sb = pp.tile([128, 2 * NQ], F16, tag="etq", name="etq")
            etp_sb = pp.tile([128, 2 * PAST_KEYS], F8, tag="etp", name="etp")
            xq_sb = pp.tile([128, 2 * NQ], F16, tag="xq", name="xq")
            xq8_sb = pp.tile([128, 2 * NQ], F8, tag="xq8", name="xq8")
            vp_sb = pp.tile([128, NPT * VW], F8, tag="vp", name="vp")
            vd_sb = pp.tile([128, NDT * VW], F16, tag="vd", name="vd")
            tri_sb = pp.tile([128, 128], F16, tag="tri", name="tri")
            bias_sb = pp.tile([128, NBIAS], F32, tag="bias", name="bias")

            def eth(t, h, c0, c1, n):
                return t[:, h * n + c0 : h * n + c1]

            def e3(t, n, c0, c1):
                """[128, 2, c1-c0] view of an h-major [128, 2n] tile."""
                return t[:].rearrange("p (h n) -> p h n", h=2)[:, :, c0:c1]

            def load_vcols(dst, src_t, c0, c1, engine):
                engine.dma_start(dst[:, c0:c1], src_t.ap()[:, c0:c1])

            def load_2h(dst, src_t, n, c0, c1, engine):
                engine.dma_start(
                    dst[:].rearrange("p (h n) -> p h n", h=2)[:, :, c0:c1],
                    src_t.ap().rearrange("p (h n) -> p h n", h=2)[:, :, c0:c1],
                )

            # PE warm-up: dense dummy matmuls on a memset tile, issued
            # before any load lands, so the HAM clock throttle ramps toward
            # full rate while the first DMAs stream in.  A dummy exp on the
            # same tile forces the ACT Exp table load off the critical path.
            wusrc = pp.tile([128, 512], F16, tag="wusrc", name="wusrc")
            zbias = pp.tile([128, 1], F32, tag="zbias", name="zbias")
            nc.vector.memset(wusrc[:], 1.0)
            nc.gpsimd.memset(zbias[:], 0.0)
            pre_t = wp.tile([128, 2], F16, tag="pre", name="pre")
            nc.scalar.activation(
                pre_t[:], wusrc[:, 0:2], mybir.ActivationFunctionType.Exp,
                bias=zbias[:, 0:1], scale=1.0,
            )
            for _ in range(NWU):
                wu = psP.tile([128, 512], F32, tag="st", name="wu")
                nc.tensor.matmul(
                    wu[:], wusrc[:, 0:128], wusrc[:],
                    start=True, stop=True,
                )

            # -------- loads, ordered by first-use; two trigger queues ------
            # Sync: chunk-0/1 ETQ + per-slot V/ETP.  Vector: W + mask/bias
            # + ETQ chunks 2-3 + slot-1 V (DVE's casts only start once
            # chunk 0 lands, so its early queue is free).  ACT: nothing.
            def load_1h(dst, src_t, n, h, c0, c1, engine):
                engine.dma_start(
                    dst[:, h * n + c0 : h * n + c1],
                    src_t.ap()[:, h * n + c0 : h * n + c1],
                )

            # The first compute (xq matmuls, then diag) is gated by W-h0 +
            # ETQ chunk-0-h0: split those loads into halves so each DMA is
            # small (lands sooner) and the halves run on parallel engines.
            nc.gpsimd.dma_start(w_sb[:, 0:D], w_d.ap()[:, 0:D])      # W h0
            load_1h(etq_sb, etq_d, NQ, 0, 0, 256, nc.sync)          # c0a h0
            nc.gpsimd.dma_start(w_sb[:, D : 2 * D],
                                w_d.ap()[:, D : 2 * D])              # W h1
            load_1h(etq_sb, etq_d, NQ, 1, 0, 256, nc.sync)          # c0a h1
            load_1h(etq_sb, etq_d, NQ, 0, 256, 512, nc.sync)        # c0b h0
            nc.gpsimd.dma_start(bias_sb[:], bias_d.ap())
            load_1h(etq_sb, etq_d, NQ, 1, 256, 512, nc.sync)        # c0b h1
            load_vcols(vd_sb, vd_d, 0, NDG * VW, nc.gpsimd)         # diag V s0
            nc.gpsimd.dma_start(tri_sb[:], tri_d.ap())
            load_vcols(vp_sb, vp_d, VP_CH[0][0] * VW, VP_CH[0][1] * VW,
                       nc.gpsimd)                                   # past V s0
            load_2h(etq_sb, etq_d, NQ, 512, 1024, nc.sync)          # chunk 1
            load_2h(etp_sb, etp_d, PAST_KEYS, *ETP_CH[0], nc.sync)  # past c0
            # slot-1 V + remaining ETQ chunks on GpSimd: Sync's early queue
            # is saturated with chunk-0/1 tensors
            load_vcols(vd_sb, vd_d, NDG * VW, 2 * NDG * VW, nc.gpsimd)
            load_vcols(vp_sb, vp_d, VP_CH[1][0] * VW, VP_CH[1][1] * VW,
                       nc.gpsimd)
            load_2h(etq_sb, etq_d, NQ, 1024, 1536, nc.gpsimd)       # chunk 2
            load_2h(etq_sb, etq_d, NQ, 1536, 2048, nc.gpsimd)       # chunk 3
            load_2h(etp_sb, etp_d, PAST_KEYS, *ETP_CH[1], nc.sync)
            for s in range(2, NSLOT):
                load_vcols(vd_sb, vd_d, s * NDG * VW, (s + 1) * NDG * VW,
                           nc.sync)
                load_2h(etp_sb, etp_d, PAST_KEYS, *ETP_CH[s], nc.sync)
                load_vcols(vp_sb, vp_d, VP_CH[s][0] * VW, VP_CH[s][1] * VW,
                           nc.sync)

            # XQ[d, q] = sum_e W[e, d] * ETQ[e, q] over query cols [qa, qb),
            # copied out both f16 (diag matmuls) and fp8 (past matmuls)
            def xq_range(qa, qb):
                # f16 casts first (they gate the next diag unit); the fp8
                # casts re-read the f16 SBUF copy so the PSUM tile frees
                # early and the diag path never queues behind them
                for dh in range(2):
                    ps = psP.tile([128, 512], F32, tag="st", name="st")
                    nc.tensor.matmul(
                        ps[:, 0 : qb - qa],
                        eth(w_sb, 0, dh * 128, (dh + 1) * 128, D),
                        eth(etq_sb, 0, qa, qb, NQ),
                        start=True,
                        stop=False,
                    )
                    nc.tensor.matmul(
                        ps[:, 0 : qb - qa],
                        eth(w_sb, 1, dh * 128, (dh + 1) * 128, D),
                        eth(etq_sb, 1, qa, qb, NQ),
                        start=False,
                        stop=True,
                    )
                    nc.vector.tensor_copy(
                        eth(xq_sb, dh, qa, qb, NQ), ps[:, 0 : qb - qa]
                    )
                for dh in range(2):
                    nc.vector.tensor_copy(
                        eth(xq8_sb, dh, qa, qb, NQ),
                        eth(xq_sb, dh, qa, qb, NQ),
                    )

            # ---------------- attention ----------------
            # Cross-slot streaming pipeline: `groups` carries
            # (matmul-args, callback) entries ACROSS slot boundaries; unit
            # j's out-matmuls are emitted right after unit j+1's score
            # matmuls, and each slot's accumulator-copy epilogue is emitted
            # as soon as its last out-matmul has actually been issued.
            groups = []

            def flush_groups(keep=0):
                while len(groups) > keep:
                    args_list, cb = groups.pop(0)
                    for args in args_list:
                        nc.tensor.matmul(*args[:3], start=args[3],
                                         stop=args[4], perf_mode=args[5])
                    if cb is not None:
                        cb()

            def mk_epilogue(s, outp):
                # copy the raw accumulators (num + den column) PSUM->SBUF,
                # store f16; host divides.
                def cb():
                    osb = sp.tile([128, NU * VW], F16, tag="osb", name="osb")
                    for u in range(NU):
                        nc.vector.tensor_copy(
                            osb[:, u * VW : (u + 1) * VW], outp[u][:]
                        )
                    nc.sync.dma_start(
                        out_d.ap()[:, s * NU * VW : (s + 1) * NU * VW],
                        osb[:],
                    )
                return cb

            # chunk 0 in two halves: slot 0 only needs XQ cols [0, 256), so
            # compute starts as soon as the first half-chunk of ETQ lands
            xq_range(0, 256)
            xq_range(256, 512)
            for s in range(NSLOT):
                q0 = s * QSTRIP
                outp = [
                    psO.tile([128, VW], F32, tag=f"outp{u}", name=f"outp{u}")
                    for u in range(NU)
                ]
                nquad = PAIRS[s] // 2
                # units: diag (2 k-tiles, fp16), quads (4 k-tiles, fp8),
                # leftover pair (2 k-tiles, fp8).  Last slot runs the diag
                # LAST so the copy/store epilogue overlaps its matmuls.
                units = (
                    [("q", i) for i in range(nquad)] + [("p", 0), ("d", 0)]
                    if s == NSLOT - 1
                    else [("d", 0)] + [("q", i) for i in range(nquad)]
                    + [("p", 0)]
                )
                pending = []
                started = [False, False]

                def out_mm(u, stat, mov, ustop):
                    pending.append((outp[u][:], stat, mov,
                                    not started[u], ustop, None))
                    started[u] = True

                def out_mm8(u, stat, mov, ustop):
                    pending.append((outp[u][:], stat, mov,
                                    not started[u], ustop, DR))
                    started[u] = True

                for j, (kind, idx) in enumerate(units):
                    last_unit = j == len(units) - 1
                    if kind == "d":
                        # ---- diag unit: 2 fp16 k-tiles, one 384-col exp
                        ps = psP.tile([128, 512], F32, tag="st", name="st")
                        for kt in range(NDG):
                            dt_i = s * NDG + kt
                            kcol = dt_i * 128
                            lo = kt * 128
                            c0 = kt * 256
                            for dh in range(2):
                                nc.tensor.matmul(
                                    ps[:, c0 : c0 + QSTRIP - lo],
                                    eth(etq_sb, dh, kcol, kcol + 128, NQ),
                                    eth(xq_sb, dh, q0 + lo, q0 + QSTRIP, NQ),
                                    start=dh == 0,
                                    stop=dh == 1,
                                )
                        flush_groups(keep=1)
                        p_t = wp.tile([128, 384], F16, tag="Pd", name="pd")
                        nc.scalar.activation(
                            p_t[:], ps[:, 0:384],
                            mybir.ActivationFunctionType.Exp,
                            bias=bias_sb[:, NSLOT : NSLOT + 1],
                            scale=INV_SQRT_D,
                        )
                        # zero the below-diagonal blocks multiplicatively on
                        # the otherwise-idle GpSimd; those out-matmuls go
                        # last so GpSimd has slack.
                        nc.gpsimd.tensor_mul(
                            p_t[:, 0:128], p_t[:, 0:128], tri_sb[:]
                        )
                        nc.gpsimd.tensor_mul(
                            p_t[:, 256:384], p_t[:, 256:384], tri_sb[:]
                        )
                        vt0 = vd_sb[:, (s * NDG) * VW : (s * NDG + 1) * VW]
                        vt1 = vd_sb[:, (s * NDG + 1) * VW : (s * NDG + 2) * VW]
                        last_s = s == NSLOT - 1
                        out_mm(1, p_t[:, 128:256], vt0, False)
                        out_mm(0, p_t[:, 0:128], vt0, last_s)
                        out_mm(1, p_t[:, 256:384], vt1, last_s)
                    elif kind == "q":
                        # ---- quad unit: pairs (2*idx, 2*idx+1) = 4 k-tiles
                        ps = psQ.tile([128, 1024], F32, tag="q", name="q")
                        for i in range(4):
                            kt = 4 * idx + i
                            nc.tensor.matmul(
                                ps[:, i * 256 : (i + 1) * 256],
                                e3(etp_sb, PAST_KEYS, kt * 128,
                                   (kt + 1) * 128),
                                e3(xq8_sb, NQ, q0, q0 + QSTRIP),
                                start=True, stop=True, perf_mode=DR,
                            )
                        flush_groups(keep=1)
                        p8 = wp.tile([128, 1024], F8, tag="Pq", name="pq")
                        nc.scalar.activation(
                            p8[:], ps[:],
                            mybir.ActivationFunctionType.Exp,
                            bias=bias_sb[:, NSLOT : NSLOT + 1],
                            scale=INV_SQRT_D,
                        )
                        for half in range(2):      # pair jp = 2*idx + half
                            kt0 = (2 * idx + half) * 2
                            mov = vp_sb[:, kt0 * VW : (kt0 + 2) * VW
                                        ].rearrange("p (h n) -> p h n", h=2)
                            pv = p8[:, half * 512 : (half + 1) * 512
                                    ].rearrange("p (h n) -> p h n", h=2)
                            for u in range(NU):
                                out_mm8(u, pv[:, :, u * 128 : (u + 1) * 128],
                                        mov, False)
                    else:
                        # ---- leftover pair: k-tiles (2jp, 2jp+1), per-core
                        # bias kills it on the half that doesn't need it
                        jp = PAIRS[s] - 1
                        ps = psP.tile([128, 512], F32, tag="st", name="st")
                        for i in range(2):
                            kt = 2 * jp + i
                            nc.tensor.matmul(
                                ps[:, i * 256 : (i + 1) * 256],
                                e3(etp_sb, PAST_KEYS, kt * 128,
                                   (kt + 1) * 128),
                                e3(xq8_sb, NQ, q0, q0 + QSTRIP),
                                start=True, stop=True, perf_mode=DR,
                            )
                        flush_groups(keep=1)
                        p8 = wp.tile([128, 512], F8, tag="Pp", name="pp8")
                        nc.scalar.activation(
                            p8[:], ps[:],
                            mybir.ActivationFunctionType.Exp,
                            bias=bias_sb[:, s : s + 1],
                            scale=INV_SQRT_D,
                        )
                        mov = vp_sb[:, 2 * jp * VW : (2 * jp + 2) * VW
                                    ].rearrange("p (h n) -> p h n", h=2)
                        pv = p8[:].rearrange("p (h n) -> p h n", h=2)
                        ustop = s != NSLOT - 1 and last_unit
                        for u in range(NU):
                            out_mm8(u, pv[:, :, u * 128 : (u + 1) * 128],
                                    mov, ustop)

                    groups.append((list(pending), None))
                    pending.clear()

                    # overlap a later slot-pair's XQ with this unit loop
                    if s in XQ_AT and j == XQ_AT[s][0]:
                        flush_groups(0)
                        c = XQ_AT[s][1]
                        xq_range(512 * c, 512 * (c + 1))

                # the slot's copy/store epilogue rides on its last unit
                groups[-1] = (groups[-1][0], mk_epilogue(s, outp))
            flush_groups(0)

    nc.compile()
    return nc


def _get_program():
    if "nc" not in _CACHE:
        _CACHE["nc"] = _build()
    return _CACHE["nc"]


# ------------------------------------------------------------- host-side data
def _hw2(x):
    """[256, N] -> SBUF-native [128, 2N] (halves side by side)."""
    return np.concatenate([x[0:128], x[128:256]], axis=1)


def _hwtiles(x):
    """[(n*128), v] -> SBUF-native [128, n*v]."""
    n = x.shape[0] // 128
    return np.ascontiguousarray(
        x.reshape(n, 128, x.shape[1]).transpose(1, 0, 2).reshape(128, -1)
    )


def _static_inputs():
    """TRI / BIAS-per-half: identical across calls."""
    if "static" in _CACHE:
        return _CACHE["static"]
    # multiplicative causal mask for the leading diag 128-block of P
    tri = np.where(
        np.arange(128)[:, None] <= np.arange(128)[None, :], 1.0, 0.0
    ).astype(np.float16)
    biases = {}
    for h in (0, 1):
        bias = np.full((128, NBIAS), EXP_BIAS, np.float32)
        for s, p in enumerate(sorted(STRIPS[h])):
            # leftover pair jp = PAIRS[s]-1 is live iff jp < p
            if not (PAIRS[s] - 1 < p):
                bias[:, s] = NEG
        biases[h] = bias
    _CACHE["static"] = (tri, biases)
    return _CACHE["static"]


def _core_inputs(encodings, W):
    tri, biases = _static_inputs()
    w = _hw2(np.asarray(W, np.float16))
    ones = np.ones((1,), NP8)
    pad = np.zeros((1,), NP8)
    in_maps = []
    for c in range(8):
        b, h = c // 2, c % 2
        e = np.asarray(encodings[b], np.float16)          # [S, D]
        et = np.ascontiguousarray(e.T)                    # [D, S]
        e8 = np.asarray(encodings[b], NP8)
        et8 = np.ascontiguousarray(e8.T)
        strips = sorted(STRIPS[h])
        etq = np.concatenate(
            [et[:, p * QSTRIP : (p + 1) * QSTRIP] for p in strips], axis=1
        )
        rows_d = np.concatenate(
            [e[p * QSTRIP : (p + 1) * QSTRIP] for p in strips], axis=0
        )
        ones16 = np.ones((1,), np.float16)
        pad16 = np.zeros((1,), np.float16)
        vd = np.concatenate(
            [rows_d, np.broadcast_to(ones16, (NQ, 1)),
             np.broadcast_to(pad16, (NQ, 1))], axis=1
        )
        vp8 = np.concatenate(
            [e8[:PAST_KEYS], np.broadcast_to(ones, (PAST_KEYS, 1)),
             np.broadcast_to(pad, (PAST_KEYS, 1))], axis=1
        )
        in_maps.append(
            {
                "ETQ": np.ascontiguousarray(_hw2(etq)),
                "ETP8": np.ascontiguousarray(_hw2(et8[:, :PAST_KEYS])),
                "VP8": _hwtiles(vp8),
                "VD": _hwtiles(vd),
                "W": np.ascontiguousarray(w),
                "TRI": tri,
                "BIAS": biases[h],
            }
        )
    return in_maps


def run_on_cores(encodings, W, trace=False, trace_cores=None):
    """Build+run; returns (output [B,S,D], BassKernelResults)."""
    _install_ntff_hook()
    nc = _get_program()
    in_maps = _core_inputs(encodings, W)
    res = bass_utils.run_bass_kernel_spmd(
        nc,
        in_maps,
        core_ids=list(range(8)),
        trace=trace,
        trace_cores=trace_cores,
    )
    out = np.empty((B, S, D), np.float32)
    for c in range(8):
        b, h = c // 2, c % 2
        o = res.results[c]["OUT"].astype(np.float32)      # [128, 16*258] f16
        o = o.reshape(128, NDT, VW).transpose(1, 0, 2).reshape(NQ, VW)
        o = o[:, 0:D] / o[:, D : D + 1]                   # host normalize
        for s, p in enumerate(sorted(STRIPS[h])):
            out[b, p * QSTRIP : (p + 1) * QSTRIP, :] = o[
                s * QSTRIP : (s + 1) * QSTRIP
            ]
    return out, res


def kernel(encodings, W):
    out, _ = run_on_cores(encodings, W, trace=False)
    return out
